# revision 11
# baseline (speedup 1.0000x reference)
"""LiteLinear (dense linear + routed LoRA) Trainium2 kernel.

out = x @ W^T + bias + scaling[aid] * ((x @ la[aid]^T) @ lb[aid]^T)   (aid>0)

Strategy: data-parallel over tokens (16384 tokens -> 2048/core on 8 cores).
Weight, LoRA stacks replicated. Per core everything fits in SBUF once:
  xt  [2048 d_in, 2048 tok]  bf16 (host-transposed, host-cast)
  wt  [2048 d_in, 2048 d_out] bf16
  lat [2048 d_in, 128 (a*r)] bf16
  lbt [128 (a*r), 2048 d_out] bf16 (scaling folded in on host)
  selt [128 (a*r), 2048 tok] f32 0/1 mask (host one-hot of lora_mapping)
  bias_r [128, 2048] f32 (bias replicated across partitions)

Device: u^T = la_all @ x^T (PE, f32 psum) ; u_m = u^T * mask (DVE, ->bf16);
main matmul accumulates 16 k-chunks into PSUM, then one extra rank-128
matmul accumulates the LoRA delta into the same PSUM bank; bias added on
DVE during PSUM->SBUF eviction; f32 DMA out.
"""

import numpy as np
import ml_dtypes

import concourse.mybir as mybir
import concourse.tile as tile
from concourse import bacc
from concourse.bass import ts
from concourse.bass_utils import run_bass_kernel_spmd

N_CORES = 8
B, S, D_IN, D_OUT = 4, 4096, 2048, 2048
N_TOK = B * S              # 16384
TOK = N_TOK // N_CORES     # 2048 tokens per core
A, R = 8, 16
AR = A * R                 # 128
P = 128
KC = D_IN // P             # 16 contraction chunks
NB = 512                   # free-dim block (one PSUM bank of f32)
GN = TOK // NB             # 4 token groups
MN = NB // P               # 4 token subtiles per group
ON = D_OUT // NB           # 4 d_out chunks

BF16 = mybir.dt.bfloat16
F32 = mybir.dt.float32

_cached_nc = None


def _build(loop_n=None):
    nc = bacc.Bacc("TRN2", target_bir_lowering=False, debug=False)
    xt = nc.dram_tensor("xt", [D_IN, TOK], BF16, kind="ExternalInput").ap()
    wt = nc.dram_tensor("wt", [D_IN, D_OUT], BF16, kind="ExternalInput").ap()
    lat = nc.dram_tensor("lat", [D_IN, AR], BF16, kind="ExternalInput").ap()
    lbt = nc.dram_tensor("lbt", [AR, D_OUT], BF16, kind="ExternalInput").ap()
    selt = nc.dram_tensor("selt", [AR, TOK], F32, kind="ExternalInput").ap()
    bias_r = nc.dram_tensor("bias_r", [P, D_OUT], F32, kind="ExternalInput").ap()
    out = nc.dram_tensor("out", [TOK, D_OUT], F32, kind="ExternalOutput").ap()

    with tile.TileContext(nc) as tc:
        with (
            tc.tile_pool(name="const", bufs=1) as cpool,
            tc.tile_pool(name="work", bufs=4) as wpool,
            tc.tile_pool(name="psum_u", bufs=2, space="PSUM") as upool,
            tc.tile_pool(name="psum_o", bufs=4, space="PSUM") as opool,
        ):
            # lat first (u-matmuls need it with chunk 0)
            lat_sb = cpool.tile([P, KC * AR], BF16, tag="lat")
            for k in range(KC):
                nc.sync.dma_start(out=lat_sb[:, ts(k, AR)],
                                  in_=lat[k * P:(k + 1) * P, :])
            # x/w chunk streams; first two chunks split 4-way so the PE's
            # first matmuls start ~3us in instead of waiting a full 1MB DMA
            xt_sb = []
            wt_sb = []
            for k in range(KC):
                spl = 1
                sr = P // spl
                xck = cpool.tile([P, TOK], BF16, tag=f"xt{k}")
                wck = cpool.tile([P, D_OUT], BF16, tag=f"wt{k}")
                for s in range(spl):
                    r0 = s * sr
                    nc.sync.dma_start(
                        out=xck[r0:r0 + sr, :],
                        in_=xt[k * P + r0:k * P + r0 + sr, :])
                    nc.sync.dma_start(
                        out=wck[r0:r0 + sr, :],
                        in_=wt[k * P + r0:k * P + r0 + sr, :])
                xt_sb.append(xck)
                wt_sb.append(wck)
            # needed only after the k-stream finishes
            lbt_sb = cpool.tile([P, D_OUT], BF16, tag="lbt")
            nc.sync.dma_start(out=lbt_sb[:], in_=lbt[:, :])
            selt_sb = cpool.tile([P, TOK], F32, tag="selt")
            nc.sync.dma_start(out=selt_sb[:], in_=selt[:, :])
            bias_sb = cpool.tile([P, D_OUT], F32, tag="bias")
            nc.sync.dma_start(out=bias_sb[:], in_=bias_r[:, :])

            def _compute():
                _emit_compute(nc, tc, wpool, upool, opool,
                              xt_sb, wt_sb, lat_sb, lbt_sb, selt_sb, bias_sb, out)

            if loop_n is None:
                _compute()
            else:
                with tc.For_i(0, loop_n, 1):
                    _compute()
    nc.compile()
    return nc


def _emit_compute(nc, tc, wpool, upool, opool,
                  xt_sb, wt_sb, lat_sb, lbt_sb, selt_sb, bias_sb, out):
    # Phase 1 (streaming): per arriving chunk k, emit all chunk-k-local work
    # so PE stays busy while x/w stream in: 4 u-group accumulations (4 PSUM
    # banks) + chunk-major accumulation of the (g0,m0) head row (4 banks).
    u_ps = [upool.tile([P, NB], F32, tag=f"u{g}", bufs=1, name=f"u{g}") for g in range(GN)]
    head_ps = [opool.tile([P, NB], F32, tag=f"o{n}", bufs=1, name=f"ho{n}") for n in range(ON)]
    for k in range(KC):
        for g in range(GN):
            nc.tensor.matmul(
                u_ps[g][:],
                lat_sb[:, ts(k, AR)],
                xt_sb[k][:, ts(g, NB)],
                start=(k == 0),
                stop=(k == KC - 1),
            )
        for n in range(ON):
            nc.tensor.matmul(
                head_ps[n][:],
                xt_sb[k][:, 0:P],
                wt_sb[k][:, ts(n, NB)],
                start=(k == 0),
                stop=False,
            )
    # mask+scale gate: u_m[g] = u[g] * mask, cast to bf16
    u_m = []
    for g in range(GN):
        um = wpool.tile([P, NB], BF16, tag=f"um{g}", bufs=1, name=f"um{g}")
        nc.vector.tensor_mul(out=um[:], in0=u_ps[g][:],
                             in1=selt_sb[:, ts(g, NB)])
        u_m.append(um)
    # finish head row: LoRA delta accumulates into same PSUM, bias on evict
    for n in range(ON):
        nc.tensor.matmul(head_ps[n][:], u_m[0][:, 0:P],
                         lbt_sb[:, ts(n, NB)], start=False, stop=True)
        o_sb = wpool.tile([P, NB], F32, tag="osb")
        nc.vector.tensor_add(out=o_sb[:], in0=head_ps[n][:],
                             in1=bias_sb[:, ts(n, NB)])
        nc.sync.dma_start(out=out[0:P, ts(n, NB)], in_=o_sb[:])
    # Phase 2: remaining 15 (g,m) rows, k-inner accumulation
    for g in range(GN):
        for m in range(MN):
            if g == 0 and m == 0:
                continue
            tok0 = g * NB + m * P
            for n in range(ON):
                o_ps = opool.tile([P, NB], F32, tag=f"o{n}", bufs=1)
                for k in range(KC):
                    nc.tensor.matmul(
                        o_ps[:],
                        xt_sb[k][:, tok0:tok0 + P],
                        wt_sb[k][:, ts(n, NB)],
                        start=(k == 0),
                        stop=False,
                    )
                nc.tensor.matmul(
                    o_ps[:],
                    u_m[g][:, ts(m, P)],
                    lbt_sb[:, ts(n, NB)],
                    start=False,
                    stop=True,
                )
                o_sb = wpool.tile([P, NB], F32, tag="osb")
                nc.vector.tensor_add(out=o_sb[:], in0=o_ps[:],
                                     in1=bias_sb[:, ts(n, NB)])
                nc.sync.dma_start(out=out[tok0:tok0 + P, ts(n, NB)],
                                  in_=o_sb[:])


def _get_nc():
    global _cached_nc
    if _cached_nc is None:
        _cached_nc = _build()
    return _cached_nc


def _prep_shared(weight, bias, lora_a, lora_b, scaling):
    bf16 = ml_dtypes.bfloat16
    wt_h = np.ascontiguousarray(np.asarray(weight, np.float32).T).astype(bf16)
    la = np.asarray(lora_a, np.float32).reshape(AR, D_IN)
    lat_h = np.ascontiguousarray(la.T).astype(bf16)
    lb = np.asarray(lora_b, np.float32) * np.asarray(scaling, np.float32)[:, None, None]
    lbt_h = np.ascontiguousarray(lb.transpose(0, 2, 1).reshape(AR, D_OUT)).astype(bf16)
    bias_h = np.ascontiguousarray(
        np.broadcast_to(np.asarray(bias, np.float32), (P, D_OUT)))
    return wt_h, lat_h, lbt_h, bias_h


def kernel(x, lora_mapping, weight, bias, lora_a, lora_b, scaling):
    bf16 = ml_dtypes.bfloat16
    nc = _get_nc()
    wt_h, lat_h, lbt_h, bias_h = _prep_shared(weight, bias, lora_a, lora_b, scaling)
    x2 = np.asarray(x, np.float32).reshape(N_TOK, D_IN)
    mapping = np.asarray(lora_mapping, np.int32)
    aid = np.arange(1, A + 1, dtype=np.int32)

    in_maps = []
    for c in range(N_CORES):
        xs = x2[c * TOK:(c + 1) * TOK]
        xt_h = np.ascontiguousarray(xs.T).astype(bf16)
        ms = mapping[c * TOK:(c + 1) * TOK]
        onehot = (ms[None, :] == aid[:, None]).astype(np.float32)   # [A, TOK]
        selt_h = np.ascontiguousarray(np.repeat(onehot, R, axis=0))  # [AR, TOK]
        in_maps.append({
            "xt": xt_h, "wt": wt_h, "lat": lat_h, "lbt": lbt_h,
            "selt": selt_h, "bias_r": bias_h,
        })

    res = run_bass_kernel_spmd(nc, in_maps, list(range(N_CORES)))
    outs = [np.asarray(res.results[c]["out"], np.float32) for c in range(N_CORES)]
    return np.concatenate(outs, axis=0).reshape(B, S, D_OUT)


# revision 12
# speedup vs baseline: 1.0130x; 1.0130x over previous
"""LiteLinear (dense linear + routed LoRA) Trainium2 kernel.

out = x @ W^T + bias + scaling[aid] * ((x @ la[aid]^T) @ lb[aid]^T)   (aid>0)

Strategy: data-parallel over tokens (16384 tokens -> 2048/core on 8 cores).
Weight, LoRA stacks replicated. Per core everything fits in SBUF once:
  xt  [2048 d_in, 2048 tok]  bf16 (host-transposed, host-cast)
  wt  [2048 d_in, 2048 d_out] bf16
  lat [2048 d_in, 128 (a*r)] bf16
  lbt [128 (a*r), 2048 d_out] bf16 (scaling folded in on host)
  selt [128 (a*r), 2048 tok] f32 0/1 mask (host one-hot of lora_mapping)
  bias_r [128, 2048] f32 (bias replicated across partitions)

Device: u^T = la_all @ x^T (PE, f32 psum) ; u_m = u^T * mask (DVE, ->bf16);
main matmul accumulates 16 k-chunks into PSUM, then one extra rank-128
matmul accumulates the LoRA delta into the same PSUM bank; bias added on
DVE during PSUM->SBUF eviction; f32 DMA out.
"""

import numpy as np
import ml_dtypes

import concourse.mybir as mybir
import concourse.tile as tile
from concourse import bacc
from concourse.bass import ts
from concourse.bass_utils import run_bass_kernel_spmd

N_CORES = 8
B, S, D_IN, D_OUT = 4, 4096, 2048, 2048
N_TOK = B * S              # 16384
TOK = N_TOK // N_CORES     # 2048 tokens per core
A, R = 8, 16
AR = A * R                 # 128
P = 128
KC = D_IN // P             # 16 contraction chunks
NB = 512                   # free-dim block (one PSUM bank of f32)
GN = TOK // NB             # 4 token groups
MN = NB // P               # 4 token subtiles per group
ON = D_OUT // NB           # 4 d_out chunks

BF16 = mybir.dt.bfloat16
F32 = mybir.dt.float32

_cached_nc = None


def _build(loop_n=None):
    nc = bacc.Bacc("TRN2", target_bir_lowering=False, debug=False)
    xt = nc.dram_tensor("xt", [D_IN, TOK], BF16, kind="ExternalInput").ap()
    wt = nc.dram_tensor("wt", [D_IN, D_OUT], BF16, kind="ExternalInput").ap()
    lat = nc.dram_tensor("lat", [D_IN, AR], BF16, kind="ExternalInput").ap()
    lbt = nc.dram_tensor("lbt", [AR, D_OUT], BF16, kind="ExternalInput").ap()
    selt = nc.dram_tensor("selt", [AR, TOK], F32, kind="ExternalInput").ap()
    bias_r = nc.dram_tensor("bias_r", [P, D_OUT], F32, kind="ExternalInput").ap()
    out = nc.dram_tensor("out", [TOK, D_OUT], F32, kind="ExternalOutput").ap()

    with tile.TileContext(nc) as tc:
        with (
            tc.tile_pool(name="const", bufs=1) as cpool,
            tc.tile_pool(name="work", bufs=4) as wpool,
            tc.tile_pool(name="psum_u", bufs=2, space="PSUM") as upool,
            tc.tile_pool(name="psum_o", bufs=4, space="PSUM") as opool,
        ):
            # lat first (u-matmuls need it with chunk 0)
            lat_sb = cpool.tile([P, KC * AR], BF16, tag="lat")
            for k in range(KC):
                nc.sync.dma_start(out=lat_sb[:, ts(k, AR)],
                                  in_=lat[k * P:(k + 1) * P, :])
            # x/w chunk streams; first two chunks split 4-way so the PE's
            # first matmuls start ~3us in instead of waiting a full 1MB DMA
            xt_sb = []
            wt_sb = []
            lbt_sb = cpool.tile([P, D_OUT], BF16, tag="lbt")
            selt_sb = cpool.tile([P, TOK], F32, tag="selt")
            bias_sb = cpool.tile([P, D_OUT], F32, tag="bias")
            for k in range(KC):
                xck = cpool.tile([P, TOK], BF16, tag=f"xt{k}")
                wck = cpool.tile([P, D_OUT], BF16, tag=f"wt{k}")
                nc.sync.dma_start(out=xck[:], in_=xt[k * P:(k + 1) * P, :])
                nc.sync.dma_start(out=wck[:], in_=wt[k * P:(k + 1) * P, :])
                xt_sb.append(xck)
                wt_sb.append(wck)
                if k == 9:
                    # mask/lbt/bias are first needed at the phase-1->head
                    # transition (~60us); issue mid-stream so they arrive
                    # just ahead of that instead of after chunk 15
                    nc.sync.dma_start(out=lbt_sb[:], in_=lbt[:, :])
                    nc.sync.dma_start(out=selt_sb[:], in_=selt[:, :])
                    nc.sync.dma_start(out=bias_sb[:], in_=bias_r[:, :])

            def _compute():
                _emit_compute(nc, tc, wpool, upool, opool,
                              xt_sb, wt_sb, lat_sb, lbt_sb, selt_sb, bias_sb, out)

            if loop_n is None:
                _compute()
            else:
                with tc.For_i(0, loop_n, 1):
                    _compute()
    nc.compile()
    return nc


def _emit_compute(nc, tc, wpool, upool, opool,
                  xt_sb, wt_sb, lat_sb, lbt_sb, selt_sb, bias_sb, out):
    # Phase 1 (streaming): per arriving chunk k, emit all chunk-k-local work
    # so PE stays busy while x/w stream in: 4 u-group accumulations (4 PSUM
    # banks) + chunk-major accumulation of the (g0,m0) head row (4 banks).
    u_ps = [upool.tile([P, NB], F32, tag=f"u{g}", bufs=1, name=f"u{g}") for g in range(GN)]
    head_ps = [opool.tile([P, NB], F32, tag=f"o{n}", bufs=1, name=f"ho{n}") for n in range(ON)]
    for k in range(KC):
        for g in range(GN):
            nc.tensor.matmul(
                u_ps[g][:],
                lat_sb[:, ts(k, AR)],
                xt_sb[k][:, ts(g, NB)],
                start=(k == 0),
                stop=(k == KC - 1),
            )
        for n in range(ON):
            nc.tensor.matmul(
                head_ps[n][:],
                xt_sb[k][:, 0:P],
                wt_sb[k][:, ts(n, NB)],
                start=(k == 0),
                stop=False,
            )
    # mask+scale gate: u_m[g] = u[g] * mask, cast to bf16
    u_m = []
    for g in range(GN):
        um = wpool.tile([P, NB], BF16, tag=f"um{g}", bufs=1, name=f"um{g}")
        nc.vector.tensor_mul(out=um[:], in0=u_ps[g][:],
                             in1=selt_sb[:, ts(g, NB)])
        u_m.append(um)
    # finish head row: LoRA delta accumulates into same PSUM, bias on evict
    for n in range(ON):
        nc.tensor.matmul(head_ps[n][:], u_m[0][:, 0:P],
                         lbt_sb[:, ts(n, NB)], start=False, stop=True)
        o_sb = wpool.tile([P, NB], F32, tag="osb")
        nc.vector.tensor_add(out=o_sb[:], in0=head_ps[n][:],
                             in1=bias_sb[:, ts(n, NB)])
        nc.sync.dma_start(out=out[0:P, ts(n, NB)], in_=o_sb[:])
    # Phase 2: remaining 15 (g,m) rows, k-inner accumulation
    for g in range(GN):
        for m in range(MN):
            if g == 0 and m == 0:
                continue
            tok0 = g * NB + m * P
            for n in range(ON):
                o_ps = opool.tile([P, NB], F32, tag=f"o{n}", bufs=1)
                for k in range(KC):
                    nc.tensor.matmul(
                        o_ps[:],
                        xt_sb[k][:, tok0:tok0 + P],
                        wt_sb[k][:, ts(n, NB)],
                        start=(k == 0),
                        stop=False,
                    )
                nc.tensor.matmul(
                    o_ps[:],
                    u_m[g][:, ts(m, P)],
                    lbt_sb[:, ts(n, NB)],
                    start=False,
                    stop=True,
                )
                o_sb = wpool.tile([P, NB], F32, tag="osb")
                nc.vector.tensor_add(out=o_sb[:], in0=o_ps[:],
                                     in1=bias_sb[:, ts(n, NB)])
                nc.sync.dma_start(out=out[tok0:tok0 + P, ts(n, NB)],
                                  in_=o_sb[:])


def _get_nc():
    global _cached_nc
    if _cached_nc is None:
        _cached_nc = _build()
    return _cached_nc


def _prep_shared(weight, bias, lora_a, lora_b, scaling):
    bf16 = ml_dtypes.bfloat16
    wt_h = np.ascontiguousarray(np.asarray(weight, np.float32).T).astype(bf16)
    la = np.asarray(lora_a, np.float32).reshape(AR, D_IN)
    lat_h = np.ascontiguousarray(la.T).astype(bf16)
    lb = np.asarray(lora_b, np.float32) * np.asarray(scaling, np.float32)[:, None, None]
    lbt_h = np.ascontiguousarray(lb.transpose(0, 2, 1).reshape(AR, D_OUT)).astype(bf16)
    bias_h = np.ascontiguousarray(
        np.broadcast_to(np.asarray(bias, np.float32), (P, D_OUT)))
    return wt_h, lat_h, lbt_h, bias_h


def kernel(x, lora_mapping, weight, bias, lora_a, lora_b, scaling):
    bf16 = ml_dtypes.bfloat16
    nc = _get_nc()
    wt_h, lat_h, lbt_h, bias_h = _prep_shared(weight, bias, lora_a, lora_b, scaling)
    x2 = np.asarray(x, np.float32).reshape(N_TOK, D_IN)
    mapping = np.asarray(lora_mapping, np.int32)
    aid = np.arange(1, A + 1, dtype=np.int32)

    in_maps = []
    for c in range(N_CORES):
        xs = x2[c * TOK:(c + 1) * TOK]
        xt_h = np.ascontiguousarray(xs.T).astype(bf16)
        ms = mapping[c * TOK:(c + 1) * TOK]
        onehot = (ms[None, :] == aid[:, None]).astype(np.float32)   # [A, TOK]
        selt_h = np.ascontiguousarray(np.repeat(onehot, R, axis=0))  # [AR, TOK]
        in_maps.append({
            "xt": xt_h, "wt": wt_h, "lat": lat_h, "lbt": lbt_h,
            "selt": selt_h, "bias_r": bias_h,
        })

    res = run_bass_kernel_spmd(nc, in_maps, list(range(N_CORES)))
    outs = [np.asarray(res.results[c]["out"], np.float32) for c in range(N_CORES)]
    return np.concatenate(outs, axis=0).reshape(B, S, D_OUT)
